# revision 29
# baseline (speedup 1.0000x reference)
"""CBAM block kernel for Trainium2, 8-core data-parallel (v7).

Computation (per image, C=256 channels, HW=56*56=3136 pixels):
  channel attention: spatial avg/max pool -> tiny MLP (BN+tanh) -> sigmoid -> ca[C]
  spatial attention: channel mean/max of ca*x -> reflect-pad 3x3 conv (2->1 ch)
                     -> two folded BNs -> sigmoid -> sa[HW]
  out = relu(sf*(x*ca*sa + x) + bfb)

v7 design (everything bf16 in SBUF, f32 for stats/scalars):
  - host interleaves x as [128, 2, HW] (partition p holds channels p, p+128);
    loads/stores are per-half DMAs on the SP queue (no stalls: loads have no
    deps, stores are emitted right after their producer)
  - consts loaded via gpsimd SWDGE so they don't delay x loads on HWDGE
  - sum pool: ACT Copy+accum in 2 chunks per half (keeps ACT queue
    interruptible); partial sums combined for free by accumulating matmuls
  - max pool: DVE tensor_scalar 4x + accum per half
  - channel max of ca*x: t0,t1 = ca_h*x_h (TS), rA = max(t0,t1) (TT),
    gpsimd partition_all_reduce; S max rows DMA'd from the Pool queue
  - channel sum: PE matmuls with cab lhsT into PSUM pieces at partition
    bases {0,32,64} x 3 tiles, 3 ACT evacs, 3 ACT-issued S row DMAs
  - conv: y-reflect folded into band matrix (host), x-reflect via 2 tiny
    gpsimd copies, 3 banded PE matmuls, sigmoid+bias ACT, saflat ACT DMA
  - sa broadcast to 128 partitions by gpsimd partition_broadcast (no DMA)
  - finals per half: M = sfca*saB+sf (TS), prod = x*M (TT),
    relu = (prod+bfb) max 0 -- relu half 0 on DVE, half 1 on ACT
"""

import os
from contextlib import ExitStack

import numpy as np

import concourse.bacc as bacc
import concourse.bass as bass
import concourse.bass_isa as bass_isa
import concourse.mybir as mybir
import concourse.tile as tile
from concourse import bass_utils

F32 = mybir.dt.float32
BF16 = mybir.dt.bfloat16
Alu = mybir.AluOpType
Act = mybir.ActivationFunctionType
AxX = mybir.AxisListType.X

B, C, H, W = 32, 256, 56, 56
HW = H * W                      # 3136
NCORES = 8
BLOC = B // NCORES              # 4 images per core
NCH = 2                         # channel chunks of 128
MID = C // 16                   # 16
APIECE = 448                    # channel-sum psum piece (1 bank of f32)
NAPIECE = HW // APIECE          # 7
SCHUNK = HW // 2                # sum-pool chunk


def _build_program(loop_k=None):
    nc = bacc.Bacc(
        "TRN2",
        target_bir_lowering=False,
        debug=False,
        enable_asserts=False,
        num_devices=NCORES,
    )

    x_d = nc.dram_tensor("x_shard", [BLOC, 128, NCH, HW], BF16,
                         kind="ExternalInput").ap()
    y_d = nc.dram_tensor("y_shard", [BLOC, 128, NCH, HW], BF16,
                         kind="ExternalOutput").ap()
    cw_d = nc.dram_tensor("cw", [128, 72], F32, kind="ExternalInput").ap()
    w2m_d = nc.dram_tensor("w2m", [MID, 258], F32, kind="ExternalInput").ap()
    bmat_d = nc.dram_tensor("bmat", [112, 168], BF16, kind="ExternalInput").ap()
    cst_d = nc.dram_tensor("conv_cst", [1, 1], F32, kind="ExternalInput").ap()
    scr_d = nc.dram_tensor("sa_scratch", [BLOC, HW], BF16, kind="Internal").ap()

    with tile.TileContext(nc) as tc:
        with ExitStack() as ctx:
            if loop_k:
                with tc.For_i(0, loop_k, 1):
                    _trace_kernel(ctx, tc, y_d, x_d, cw_d, w2m_d, bmat_d,
                                  cst_d, scr_d)
            else:
                _trace_kernel(ctx, tc, y_d, x_d, cw_d, w2m_d, bmat_d, cst_d,
                              scr_d)
    nc.compile()
    return nc


def _trace_kernel(ctx, tc, y_d, x_d, cw_d, w2m_d, bmat_d, cst_d, scr_d):
    nc = tc.nc
    BCAST_DMA = {0, 1}          # images whose sa broadcast goes via DRAM/DMA
    SUM_DVE = {2, 3}            # images whose sum pool runs on DVE

    consts = ctx.enter_context(tc.tile_pool(name="consts", bufs=1))
    pxb = ctx.enter_context(tc.tile_pool(name="pxb", bufs=BLOC))
    ptrash = ctx.enter_context(tc.tile_pool(name="ptrash", bufs=2))
    pstat = ctx.enter_context(tc.tile_pool(name="pstat", bufs=1))
    pfull = ctx.enter_context(tc.tile_pool(name="pfull", bufs=2))
    psmall = ctx.enter_context(tc.tile_pool(name="psmall", bufs=2))
    pout = ctx.enter_context(tc.tile_pool(name="pout", bufs=2))
    ps_mlp = ctx.enter_context(tc.tile_pool(name="ps_mlp", bufs=2, space="PSUM"))
    ps_csA = ctx.enter_context(tc.tile_pool(name="ps_csA", bufs=2, space="PSUM"))
    ps_csB = ctx.enter_context(tc.tile_pool(name="ps_csB", bufs=2, space="PSUM"))
    ps_one = ctx.enter_context(tc.tile_pool(name="ps_one", bufs=1, space="PSUM"))

    # ---- preload both ACT function tables off the critical path ----
    dum = consts.tile([1, 1], F32, tag="dum")
    dtr = consts.tile([1, 1], F32, tag="dtr")
    nc.vector.memset(dum, 0.0)
    for fn in (Act.Copy, Act.Tanh, Act.Sigmoid, Act.Relu):
        nc.scalar.activation(out=dtr, in_=dum, func=fn)

    # ---- constants via SWDGE (Pool) so HWDGE stays free for x loads ----
    cw = consts.tile([128, 72], F32, tag="cw")
    nc.gpsimd.dma_start(out=cw, in_=cw_d)
    w2m = consts.tile([MID, 258], F32, tag="w2m")
    nc.gpsimd.dma_start(out=w2m, in_=w2m_d)
    bmat = consts.tile([112, 168], BF16, tag="bmat")
    nc.gpsimd.dma_start(out=bmat, in_=bmat_d)
    cst56 = consts.tile([56, 1], F32, tag="cst56")
    nc.gpsimd.dma_start(out=cst56, in_=cst_d.to_broadcast((56, 1)))

    w1a = [cw[:, 0:16], cw[:, 16:32]]
    w1m = [cw[:, 32:48], cw[:, 48:64]]
    chv = [cw[:, 64:68], cw[:, 68:72]]
    mlpv = w2m[:, 0:2]
    w2t = w2m[:, 2:258]

    # ---- per-image state ----
    xb = {}
    ssum, smax = {}, {}
    ca, cab, sfca = {}, {}, {}
    Ss, saBs, ybig = {}, {}, {}
    saRows = {}
    CH2 = HW // 2               # column half for chmax/bcast/finals split

    def emit_load(i, h):
        if (i, h) == (0, 0):
            for j in range(BLOC):
                xb[j] = pxb.tile([128, NCH, HW], BF16, tag="xb",
                                 name=f"xb{j}")
        nc.sync.dma_start(out=xb[i][:, h, :], in_=x_d[i, :, h, :])

    def emit_pool_sum(i, h, k):
        # ACT images: chunk k of half h (Copy + accum), partials combined by
        # accumulating matmuls.  DVE images: one TS+accum per half (k == 0).
        if (h, k) == (0, 0):
            nk = 1 if i in SUM_DVE else 2
            ssum[i] = [[pstat.tile([128, 1], F32, tag=f"ssum{hh}{kk}_{i}",
                                   name=f"ssum{hh}{kk}_{i}")
                        for kk in range(nk)] for hh in range(NCH)]
        if i in SUM_DVE:
            if k > 0:
                return
            trS = ptrash.tile([128, HW], BF16, tag="trB", name="trS")
            nc.vector.tensor_scalar(out=trS, in0=xb[i][:, h, :], scalar1=1.0,
                                    scalar2=None, op0=Alu.mult, op1=Alu.add,
                                    accum_out=ssum[i][h][0])
        else:
            trA = ptrash.tile([128, SCHUNK], BF16, tag="trA")
            nc.scalar.activation(out=trA,
                                 in_=xb[i][:, h, k * SCHUNK:(k + 1) * SCHUNK],
                                 func=Act.Copy, accum_out=ssum[i][h][k])

    def emit_pool_max(i, h):
        if h == 0:
            smax[i] = [pstat.tile([128, 1], F32, tag=f"smax{hh}_{i}",
                                  name=f"smax{hh}_{i}") for hh in range(NCH)]
        trB = ptrash.tile([128, HW], BF16, tag="trB")
        nc.vector.tensor_scalar(out=trB, in0=xb[i][:, h, :], scalar1=1.0,
                                scalar2=None, op0=Alu.mult, op1=Alu.max,
                                accum_out=smax[i][h])

    def emit_mlp(i):
        mlpt = ps_mlp.tile([128, 2], F32, tag="mlp", name=f"mlp1_{i}")
        mlp1 = mlpt[0:MID, :]
        nk = len(ssum[i][0])
        for h in range(NCH):
            for k in range(nk):
                nc.tensor.matmul(out=mlp1[:, 0:1], lhsT=w1a[h],
                                 rhs=ssum[i][h][k],
                                 start=(h == 0 and k == 0),
                                 stop=(h == 1 and k == nk - 1))
        for h in range(NCH):
            nc.tensor.matmul(out=mlp1[:, 1:2], lhsT=w1m[h], rhs=smax[i][h],
                             start=(h == 0), stop=(h == 1))
        th1 = pstat.tile([MID, 2], F32, tag=f"th1_{i % 2}", name=f"th1_{i}")
        nc.scalar.activation(out=th1, in_=mlp1, func=Act.Tanh,
                             bias=mlpv[:, 1:2], scale=mlpv[:, 0:1])
        ca[i], cab[i], sfca[i] = [], [], []
        for h in range(NCH):
            mlp2 = ps_mlp.tile([128, 2], F32, tag="mlp", name=f"mlp2_{i}{h}")
            nc.tensor.matmul(out=mlp2[:, 0:1], lhsT=w2t[:, h * 128:(h + 1) * 128],
                             rhs=th1[:, 0:1], start=True, stop=True)
            nc.tensor.matmul(out=mlp2[:, 1:2], lhsT=w2t[:, h * 128:(h + 1) * 128],
                             rhs=th1[:, 1:2], start=True, stop=True)
            th2 = pstat.tile([128, 2], F32, tag=f"th2_{i % 2}{h}",
                             name=f"th2_{i}{h}")
            nc.scalar.activation(out=th2, in_=mlp2, func=Act.Tanh,
                                 bias=chv[h][:, 1:2], scale=chv[h][:, 0:1])
            # ca = sigmoid(th2[:,0] + th2[:,1]) fused via ACT bias
            cat = pstat.tile([128, 1], F32, tag=f"ca_{i % 2}{h}",
                             name=f"ca_{i}{h}")
            nc.scalar.activation(out=cat, in_=th2[:, 0:1], func=Act.Sigmoid,
                                 bias=th2[:, 1:2], scale=1.0)
            ca[i].append(cat)
            cb = pstat.tile([128, 1], BF16, tag=f"cab_{i % 2}{h}",
                            name=f"cab_{i}{h}")
            nc.scalar.activation(out=cb, in_=cat, func=Act.Copy)
            cab[i].append(cb)
            sf = pstat.tile([128, 1], F32, tag=f"sfca_{i % 2}{h}",
                            name=f"sfca_{i}{h}")
            nc.scalar.activation(out=sf, in_=cat, func=Act.Copy,
                                 scale=chv[h][:, 2:3])
            sfca[i].append(sf)

    def emit_chmax_dve(i, c):
        sl = slice(c * CH2, (c + 1) * CH2)
        t0 = pfull.tile([128, CH2], BF16, tag=f"t0c{c}")
        nc.vector.tensor_scalar(out=t0, in0=xb[i][:, 0, sl], scalar1=ca[i][0],
                                scalar2=None, op0=Alu.mult)
        t1 = pfull.tile([128, CH2], BF16, tag=f"t1c{c}")
        nc.vector.tensor_scalar(out=t1, in0=xb[i][:, 1, sl], scalar1=ca[i][1],
                                scalar2=None, op0=Alu.mult)
        rA = pfull.tile([128, CH2], BF16, tag=f"rAc{c}")
        nc.vector.tensor_tensor(out=rA, in0=t0, in1=t1, op=Alu.max)
        return rA

    rmaxs = {}

    def emit_chmax_pool(i, c, rA):
        if c == 0:
            Ss[i] = psmall.tile([112, 58], BF16, tag="S", name=f"S_{i}")
            rmaxs[i] = pfull.tile([128, HW], BF16, tag="rmax", name=f"rmax_{i}")
        rmax = rmaxs[i]
        nc.gpsimd.partition_all_reduce(rmax[:, c * CH2:(c + 1) * CH2], rA,
                                       channels=128,
                                       reduce_op=bass_isa.ReduceOp.max)
        if c == 1:
            # one DMA for all 56 max rows; stalls Pool queue only briefly
            # since the second all_reduce just finished
            nc.gpsimd.dma_start(out=Ss[i][56:112, 1:57], in_=rmax[0:1, :])

    def emit_chsum(i):
        psA = ps_csA.tile([65, APIECE], F32, tag="csA", name=f"csA_{i}")
        psB = ps_csB.tile([65, APIECE], F32, tag="csB", name=f"csB_{i}")
        psC = ps_one.tile([1, APIECE], F32, tag="csC", name=f"csC_{i}")
        for h in range(NCH):
            for p in range(NAPIECE):
                at = psA if p < 3 else (psB if p < 6 else psC)
                base = 32 * (p % 3) if p < 6 else 0
                sl = slice(p * APIECE, (p + 1) * APIECE)
                nc.tensor.matmul(out=at[base:base + 1, :], lhsT=cab[i][h],
                                 rhs=xb[i][:, h, sl],
                                 start=(h == 0), stop=(h == 1))
        S = Ss[i]
        av = psmall.tile([65, 3 * APIECE], BF16, tag="av", name=f"av_{i}")
        nc.scalar.activation(out=av[:, 0:APIECE], in_=psA, func=Act.Copy)
        nc.scalar.activation(out=av[:, APIECE:2 * APIECE], in_=psB,
                             func=Act.Copy)
        nc.scalar.activation(out=av[0:1, 2 * APIECE:3 * APIECE], in_=psC,
                             func=Act.Copy)
        # rows 0..23 pieces 0-2, rows 24..47 pieces 3-5, rows 48..55 piece 6
        nc.scalar.dma_start(
            out=S[0:24, 1:57],
            in_=av[0:65:32, 0:APIECE].rearrange("p (j x) -> p j x", j=8))
        nc.scalar.dma_start(
            out=S[24:48, 1:57],
            in_=av[0:65:32, APIECE:2 * APIECE].rearrange("p (j x) -> p j x",
                                                         j=8))
        nc.scalar.dma_start(out=S[48:56, 1:57],
                            in_=av[0:1, 2 * APIECE:3 * APIECE])

    def emit_conv(i):
        S = Ss[i]
        nc.gpsimd.tensor_copy(out=S[:, 0:1], in_=S[:, 2:3])
        nc.gpsimd.tensor_copy(out=S[:, 57:58], in_=S[:, 55:56])
        conv = ps_one.tile([56, 56], F32, tag="conv", name=f"conv_{i}")
        for dx in range(3):
            nc.tensor.matmul(out=conv, lhsT=bmat[:, dx * 56:(dx + 1) * 56],
                             rhs=S[:, dx:dx + 56],
                             start=(dx == 0), stop=(dx == 2))
        sa56 = psmall.tile([56, 56], BF16, tag="sa56", name=f"sa56_{i}")
        nc.scalar.activation(out=sa56, in_=conv[0:56, 0:56], func=Act.Sigmoid,
                             bias=cst56, scale=1.0)
        if i in BCAST_DMA:
            nc.scalar.dma_start(out=scr_d[i:i + 1, :], in_=sa56)
        else:
            saRow = psmall.tile([1, HW], BF16, tag="saRow", name=f"saRow_{i}")
            nc.scalar.dma_start(out=saRow, in_=sa56)
            saRows[i] = saRow

    def emit_bcast(i, c):
        if c == 0:
            saBs[i] = []
        if i in BCAST_DMA:
            if c == 1:
                return      # single full-width DMA emitted at c == 0
            saB = pfull.tile([128, HW], BF16, tag="saBdma", name=f"saBdma_{i}")
            nc.sync.dma_start(out=saB,
                              in_=scr_d[i:i + 1, :].to_broadcast((128, HW)))
            saBs[i] = [saB[:, 0:CH2], saB[:, CH2:HW]]
        else:
            saB = pfull.tile([128, CH2], BF16, tag=f"saBc{c}")
            nc.gpsimd.partition_broadcast(saB,
                                          saRows[i][:, c * CH2:(c + 1) * CH2])
            saBs[i].append(saB)

    def emit_final(i, h, c, relu_act=False):
        # one quadrant (channel half h, column half c)
        if (h, c) == (0, 0):
            ybig[i] = [[pout.tile([128, CH2], BF16, tag=f"y{hh}{cc}",
                                  name=f"y{hh}{cc}_{i}")
                        for cc in range(2)] for hh in range(NCH)]
        sl = slice(c * CH2, (c + 1) * CH2)
        M = pfull.tile([128, CH2], BF16, tag=f"Mc{c}")
        nc.vector.tensor_scalar(out=M, in0=saBs[i][c], scalar1=sfca[i][h],
                                scalar2=chv[h][:, 2:3], op0=Alu.mult,
                                op1=Alu.add)
        yq = ybig[i][h][c]
        nc.vector.tensor_tensor(out=yq, in0=xb[i][:, h, sl], in1=M,
                                op=Alu.mult)
        if relu_act:
            nc.scalar.activation(out=yq, in_=yq, func=Act.Relu,
                                 bias=chv[h][:, 3:4], scale=1.0)
        else:
            nc.vector.tensor_scalar(out=yq, in0=yq, scalar1=chv[h][:, 3:4],
                                    scalar2=0.0, op0=Alu.add, op1=Alu.max)
        nc.sync.dma_start(out=y_d[i, :, h, sl], in_=yq)

    # ---- emission ----
    for i in range(BLOC):
        for h in range(NCH):
            emit_load(i, h)

    def stage_pool(i):
        emit_pool_sum(i, 0, 0)
        emit_pool_max(i, 0)
        emit_pool_sum(i, 0, 1)
        emit_pool_sum(i, 1, 0)
        emit_pool_max(i, 1)
        emit_pool_sum(i, 1, 1)

    def stage_chmax_dve(i):
        return [emit_chmax_dve(i, 0), emit_chmax_dve(i, 1)]

    def stage_chmax_pool(i, rAs):
        emit_chmax_pool(i, 0, rAs[0])
        emit_chmax_pool(i, 1, rAs[1])

    def stage_bcast(i):
        emit_bcast(i, 0)
        emit_bcast(i, 1)

    def stage_final(i, c):
        for h in range(NCH):
            emit_final(i, h, c, relu_act=(h == 1))

    rAs = {}
    order = [
        ("pool", 0),
        ("mlp", 0),
        ("pool", 1),
        ("mlp", 1),
        ("chmaxD", 0),
        ("chmaxP", 0),
        ("chsum", 0),
        ("pool", 2),
        ("mlp", 2),
        ("chmaxD", 1),
        ("chmaxP", 1),
        ("chsum", 1),
        ("pool", 3),
        ("mlp", 3),
        ("conv", 0),
        ("chmaxD", 2),
        ("bcast", 0),
        ("chmaxP", 2),
        ("chsum", 2),
        ("conv", 1),
        ("chmaxD", 3),
        ("bcast", 1),
        ("chmaxP", 3),
        ("chsum", 3),
        ("final", 0, 0),
        ("conv", 2),
        ("final", 0, 1),
        ("bcast", 2),
        ("final", 1, 0),
        ("conv", 3),
        ("final", 1, 1),
        ("bcast", 3),
        ("final", 2, 0),
        ("final", 2, 1),
        ("final", 3, 0),
        ("final", 3, 1),
    ]

    for item in order:
        kind = item[0]
        if kind == "pool":
            stage_pool(item[1])
        elif kind == "mlp":
            emit_mlp(item[1])
        elif kind == "chmaxD":
            rAs[item[1]] = stage_chmax_dve(item[1])
        elif kind == "chmaxP":
            stage_chmax_pool(item[1], rAs[item[1]])
        elif kind == "chsum":
            emit_chsum(item[1])
        elif kind == "conv":
            emit_conv(item[1])
        elif kind == "bcast":
            stage_bcast(item[1])
        elif kind == "final":
            stage_final(item[1], item[2])


# ---------------------------------------------------------------------------
# host-side parameter folding
# ---------------------------------------------------------------------------

def _fold_params(inp):
    f = lambda a: np.asarray(a, dtype=np.float32)
    import ml_dtypes

    s1 = f(inp["bn1_g"]) / np.sqrt(f(inp["bn1_v"]) + 1e-5)
    b1 = f(inp["bn1_b"]) - f(inp["bn1_m"]) * s1
    s2 = f(inp["bn2_g"]) / np.sqrt(f(inp["bn2_v"]) + 1e-5)
    b2 = f(inp["bn2_b"]) - f(inp["bn2_m"]) * s2
    sf = f(inp["fbn_g"]) / np.sqrt(f(inp["fbn_v"]) + 1e-5)
    bfb = f(inp["fbn_b"]) - f(inp["fbn_m"]) * sf

    w1 = f(inp["w1"])                      # [MID, C]
    w2 = f(inp["w2"])                      # [C, MID]
    w1t_avg = np.ascontiguousarray((w1 / HW).T)     # [C, MID]
    w1t_max = np.ascontiguousarray(w1.T)            # [C, MID]
    w2t = np.ascontiguousarray(w2.T)                # [MID, C]
    mlp_vec = np.stack([s1, b1], axis=1)            # [MID, 2]
    ch_vec = np.stack([s2, b2, sf, bfb], axis=1)    # [C, 4]

    cw = np.zeros((128, 72), np.float32)
    cw[:, 0:16] = w1t_avg[0:128]
    cw[:, 16:32] = w1t_avg[128:256]
    cw[:, 32:48] = w1t_max[0:128]
    cw[:, 48:64] = w1t_max[128:256]
    cw[:, 64:68] = ch_vec[0:128]
    cw[:, 68:72] = ch_vec[128:256]

    w2m = np.zeros((MID, 258), np.float32)
    w2m[:, 0:2] = mlp_vec
    w2m[:, 2:258] = w2t

    a1 = f(inp["sbn1_g"])[0] / np.sqrt(f(inp["sbn1_v"])[0] + 1e-3)
    c1 = f(inp["sbn1_b"])[0] - f(inp["sbn1_m"])[0] * a1
    a2 = f(inp["sbn2_g"])[0] / np.sqrt(f(inp["sbn2_v"])[0] + 1e-5)
    c2 = f(inp["sbn2_b"])[0] - f(inp["sbn2_m"])[0] * a2
    amul = a1 * a2
    cst = a2 * (a1 * f(inp["sconv_b"])[0] + c1) + c2

    wsp = f(inp["sconv_w"])[0]             # [2, 3, 3]
    w_eff = np.stack([wsp[0] * amul / C, wsp[1] * amul])  # [2(ic), 3(dy), 3(dx)]

    # band matrix with y-reflect folded in: S is [112, 58] (56 avg + 56 max
    # rows, no y pad); bmat[ic*56 + yy, dx*56 + y] sums w_eff over taps
    bmat = np.zeros((112, 168), np.float32)
    for ic in range(2):
        for dx in range(3):
            for y in range(56):
                for dy in range(3):
                    yy = y - 1 + dy
                    if yy < 0:
                        yy = -yy
                    elif yy > 55:
                        yy = 110 - yy
                    bmat[ic * 56 + yy, dx * 56 + y] += w_eff[ic, dy, dx]

    return {
        "cw": cw, "w2m": w2m,
        "bmat": bmat.astype(ml_dtypes.bfloat16),
        "conv_cst": np.full((1, 1), cst, np.float32),
    }


_NC_CACHE = {}


def _get_program():
    if "nc" not in _NC_CACHE:
        _NC_CACHE["nc"] = _build_program()
    return _NC_CACHE["nc"]


def make_in_maps(inputs):
    import ml_dtypes
    params = _fold_params(inputs)
    x = np.asarray(inputs["x"], dtype=np.float32).reshape(B, NCH, 128, HW)
    xb = x.astype(ml_dtypes.bfloat16).transpose(0, 2, 1, 3)  # [B,128,2,HW]
    return [{"x_shard": np.ascontiguousarray(xb[core * BLOC:(core + 1) * BLOC]),
             **params} for core in range(NCORES)]


def kernel(**inputs) -> np.ndarray:
    nc = _get_program()
    in_maps = make_in_maps(inputs)
    res = bass_utils.run_bass_kernel_spmd(nc, in_maps, core_ids=list(range(NCORES)))
    out = np.concatenate(
        [r["y_shard"].astype(np.float32).transpose(0, 2, 1, 3).reshape(BLOC, C, HW)
         for r in res.results], axis=0)
    return out.reshape(B, C, H, W)
